# revision 70
# baseline (speedup 1.0000x reference)
"""CondConv2d on 8 Trainium2 NeuronCores — data-parallel over batch N=8.

v3 design (per core, one sample):
  - bf16 end-to-end on the wire; static residual conv folded into the
    expert banks host-side (W'_k = W_k + conv_w), softmax handled by
    normalizing exp(logits) before the weight mix.
  - Dual-plane conv: the PE's full 128 output columns = 64 channels x 2
    output-row planes.  Window (a, c) streams x rows [a, a+2] at column
    offset c; plane A (cols 0-63) computes out rows a+q from taps
    (0,c) [lower ci] + (+1,c) [row-shifted upper ci]; plane B (cols
    64-127) computes out rows a+q+1 from tap (-1,c) [lower ci].  All 9
    taps in 3 matmuls per 3-row tile (vs 6 for the single-plane form).
  - Eviction adds the two planes (plane B of tile j-1 supplies row 3j)
    plus conv bias, strips the 130-wide padding, writes packed bf16
    staging; one out-DMA per 4 tiles.
  - x lower copy is split across both hwdge queues and lands first
    (gates the attention sums); weight banks follow on sync; the
    row-shifted upper copy streams last in 16-row chunks, staying ahead
    of the conv's consumption.
"""
import os
import numpy as np

N, C, H, W = 8, 64, 128, 128
K = 4
WP = W + 2                 # padded row width (130)
NELEM = WP * WP + 2        # per-partition x buffer length (16902)
RPT = 3                    # output rows per PSUM tile
HWELEM = H * WP            # 16640
F3 = WP * RPT              # stream/psum free size (390)

# lower-copy chunks (elements per partition).  Rows 0-8 and 96-128 land
# first so every edge/corner reduction depends only on early chunks (a
# late-dep op at the head of the in-order DVE stream would block all
# span folds behind it).  sync carries 76 rows, scalar 52 (its queue
# starts ~2.4us later).
L_SYNC = [(0, 1040), (12480, 4160), (1040, 2600), (3640, 1040)]  # r0-8, r96-128, r8-28, r28-36
L_SCAL = [(4680, 3640), (8320, 2600), (10920, 1560)]  # r36-64, r64-84, r84-96
# upper-copy 16-row chunks: sync issues most of them (its engine has no
# compute duties, so ring-full stalls on dma_start are harmless); scalar
# issues two late ones after its attention work
U_SYNC = [(0, 2080), (2080, 2080), (4160, 2080), (6240, 2080),
          (8320, 2080), (12480, 2080)]
U_SCAL = [(10400, 2080), (14560, 2080)]


# ----------------------------------------------------------------------------
# host-side prep
# ----------------------------------------------------------------------------
def _make_cw2(net0_w, net0_b, net1_w, net1_b, net2_w, net2_b):
    """CW2[c, b, k]: logits[k] = sum_{c,b} CW2[c,b,k] * basis[c,b].
    basis: 0=total, 1=row0, 2=row127, 3=col0, 4=col127,
           5..8=corners (00,0W,H0,HW), 9=const 1."""
    cw = np.zeros((C, 10, K), np.float64)
    scale = 1.0 / (C * H * W)
    for w_net, pads in ((net0_w, (0, 0, 0)), (net1_w, (1, 1, 1)), (net2_w, (2, 1, 1))):
        Kk, _, kd, kh, kw = w_net.shape
        pd, ph, pw = pads
        for i in range(kd):
            clo, chi = max(0, i - pd), min(C - 1, C - 1 + i - pd)
            cmask = np.zeros(C)
            cmask[clo:chi + 1] = 1.0
            for j in range(kh):
                hlo, hhi = max(0, j - ph), min(H - 1, H - 1 + j - ph)
                dropA = 0 if hlo == 1 else (127 if hhi == H - 2 else None)
                for l in range(kw):
                    wlo, whi = max(0, l - pw), min(W - 1, W - 1 + l - pw)
                    dropB = 0 if wlo == 1 else (127 if whi == W - 2 else None)
                    v = np.zeros(10)
                    v[0] = 1.0
                    if dropA == 0: v[1] = -1.0
                    if dropA == 127: v[2] = -1.0
                    if dropB == 0: v[3] = -1.0
                    if dropB == 127: v[4] = -1.0
                    if dropA is not None and dropB is not None:
                        v[{(0, 0): 5, (0, 127): 6, (127, 0): 7, (127, 127): 8}[(dropA, dropB)]] = 1.0
                    for k in range(Kk):
                        cw[:, :, k] += w_net[k, 0, i, j, l] * scale * np.outer(cmask, v)
    btot = (net0_b + net1_b + net2_b).astype(np.float64)
    cw[:, 9, :] += btot[None, :] / C
    return np.ascontiguousarray(cw.astype(np.float32))


def _make_bank3(Wt):
    """Wt (co, ci, 3, 3) -> (128, 3, 128) dual-plane stationary layout.
    Window w (col offset c=w-1): rows 0-63 = lower ci, rows 64-127 =
    row-shifted upper ci; cols 0-63 = plane A (out row a+q), cols
    64-127 = plane B (out row a+q+1)."""
    bank = np.zeros((128, 3, 128), np.float32)
    for w in range(3):
        bank[:64, w, 0:64] = Wt[:, :, 1, w].T     # A: tap (0, c)
        bank[64:, w, 0:64] = Wt[:, :, 2, w].T     # A: tap (+1, c)
        bank[:64, w, 64:128] = Wt[:, :, 0, w].T   # B: tap (-1, c)
    return bank


# ----------------------------------------------------------------------------
# device program
# ----------------------------------------------------------------------------
_NC_CACHE = {}


def _build_nc():
    import concourse.bacc as bacc
    import concourse.tile as tile
    from concourse import mybir

    f32 = mybir.dt.float32
    bf16 = mybir.dt.bfloat16
    Alu = mybir.AluOpType
    Ax = mybir.AxisListType
    Act = mybir.ActivationFunctionType

    nc = bacc.Bacc("TRN2", target_bir_lowering=False, debug=False,
                   enable_asserts=False, num_devices=N)
    xin = nc.dram_tensor("xin", [C, HWELEM], bf16, kind="ExternalInput")
    # weight banks [128, K, 384] + smalls (cw2 40 + convb 1 on partitions
    # 0-63 in the last 48 cols); loaded via gpsimd's software DGE so it
    # doesn't compete with the x stream on the hwdge queues
    wbs = nc.dram_tensor("wbs", [128, K * 384 + 48], bf16, kind="ExternalInput")
    # flat position-indexed output (host reshapes to rows and strips the
    # 130-wide padding) so evictions and out-DMAs are fully contiguous
    outT = nc.dram_tensor("out", [C, 16772], bf16, kind="ExternalOutput")

    with tile.TileContext(nc) as tc:
        with tc.tile_pool(name="singles", bufs=1) as S, \
             tc.tile_pool(name="stage", bufs=4) as STG, \
             tc.tile_pool(name="cpsum", bufs=6, space="PSUM") as PS, \
             tc.tile_pool(name="spsum", bufs=1, space="PSUM") as PS1:

            XL = S.tile([128, NELEM], bf16)
            wb_sb = S.tile([128, K * 384 + 48], bf16)
            zrow = S.tile([128, 128], bf16)       # zeros, warmup lhs
            onesall = S.tile([C, 128], bf16)      # ones, logits broadcast
            att_sb = S.tile([128, K], f32)        # exp(logits)
            attn = S.tile([128, K], f32)          # normalized attention
            M10 = S.tile([C, 10], f32)
            PARTS = S.tile([C, 14], f32)
            COLP = S.tile([C, 14], f32)
            G = S.tile([C, K], f32)
            G_bf = S.tile([C, K], bf16)
            convb128 = S.tile([128, 1], f32)
            mw = S.tile([128, 3, 128], f32)
            mwb = S.tile([128, 3, 128], bf16)
            bplane = S.tile([64, 16904], f32)     # B-plane staging by position
            scr_d = S.tile([C, 2600], bf16)       # DVE fold scratch
            scr_a = S.tile([C, 2600], bf16)       # ACT span scratch
            scr_g = S.tile([C, 16], f32)          # G contraction scratch
            rs_sum = S.tile([128, 1], f32)
            rs_inv = S.tile([128, 1], f32)

            wpsum = PS1.tile([128, 512], f32)
            psum_b = PS1.tile([128, K], f32)

            cw2v = wb_sb[0:64, K * 384:K * 384 + 40].rearrange(
                "p (b k) -> p b k", k=K)

            # --- constants / border zeroing ---
            nc.vector.memset(zrow, 0.0)
            nc.vector.memset(onesall, 1.0)
            nc.vector.memset(M10[:, 9:10], 1.0)
            nc.vector.memset(XL[0:64, 0:132], 0.0)
            nc.vector.memset(XL[0:64, 132 + HWELEM:NELEM], 0.0)
            nc.vector.memset(XL[64:128, 0:2], 0.0)
            nc.vector.memset(XL[64:128, 2 + HWELEM:NELEM], 0.0)

            # --- input DMAs (queue order == issue order == emission order) ---
            nc.gpsimd.dma_start(out=wb_sb, in_=wbs[:, :])
            for a, ln in L_SYNC:
                nc.sync.dma_start(out=XL[0:64, 132 + a:132 + a + ln],
                                  in_=xin[:, a:a + ln])
            for a, ln in L_SCAL:
                nc.scalar.dma_start(out=XL[0:64, 132 + a:132 + a + ln],
                                    in_=xin[:, a:a + ln])
            for a, ln in U_SYNC:
                nc.sync.dma_start(out=XL[64:128, 2 + a:2 + a + ln],
                                  in_=xin[:, a:a + ln])
            # (U_SCAL issues are emitted after the ACT attention work below)

            # --- PE warm-up (results discarded; zrow is all-zero) ---
            for i in range(8):
                nc.tensor.matmul(wpsum[:, 0:128], zrow, zrow, start=True, stop=True)

            # --- attention basis sums, per lower chunk, in landing order.
            # Each chunk also contributes its own col0/col127 partials so no
            # strided reduce depends on a late chunk. ---
            lchunks = [L_SYNC[0], L_SYNC[1], L_SCAL[0], L_SYNC[2],
                       L_SCAL[1], L_SYNC[3], L_SCAL[2]]
            for c, (a0, ln) in enumerate(lchunks):
                a = 132 + a0
                dl = (ln * 5 // 8) & ~1          # DVE share (even)
                h = dl // 2
                nc.vector.scalar_tensor_tensor(
                    out=scr_d[:, :h], in0=XL[0:64, a:a + h], scalar=1.0,
                    in1=XL[0:64, a + h:a + dl], op0=Alu.mult, op1=Alu.add,
                    accum_out=PARTS[:, c:c + 1])
                nc.scalar.activation(
                    out=scr_a[:, :ln - dl], in_=XL[0:64, a + dl:a + ln],
                    func=Act.Identity, bias=0.0, scale=1.0,
                    accum_out=PARTS[:, 7 + c:8 + c])
                cv0 = XL[0:64, a:a + ln].rearrange("p (r w) -> p r w", w=WP)
                nc.vector.tensor_reduce(out=COLP[:, c:c + 1],
                                        in_=cv0[:, :, 0:1], axis=Ax.XY, op=Alu.add)
                nc.vector.tensor_reduce(out=COLP[:, 7 + c:8 + c],
                                        in_=cv0[:, :, 127:128], axis=Ax.XY, op=Alu.add)
                if c == 0:
                    # rows 0-8 chunk: row-0 sum + top corners
                    nc.vector.tensor_reduce(out=M10[:, 1:2],
                                            in_=XL[0:64, 132:132 + W],
                                            axis=Ax.X, op=Alu.add)
                    nc.vector.tensor_copy(
                        out=M10[:, 5:7].rearrange("p (a b) -> p a b", b=1),
                        in_=XL[0:64, 132:132 + 254].rearrange(
                            "p (a b) -> p a b", b=127)[:, :, 0:1])
                if c == 1:
                    # rows 96-128 chunk: row-127 sum + bottom corners
                    nc.vector.tensor_reduce(out=M10[:, 2:3],
                                            in_=XL[0:64, 16642:16642 + W],
                                            axis=Ax.X, op=Alu.add)
                    nc.vector.tensor_copy(
                        out=M10[:, 7:9].rearrange("p (a b) -> p a b", b=1),
                        in_=XL[0:64, 16642:16642 + 254].rearrange(
                            "p (a b) -> p a b", b=127)[:, :, 0:1])
                # keep the PE clock ramped: dummy matmul gated on this chunk
                nc.tensor.matmul(wpsum[:, 0:512], zrow[0:64, :],
                                 XL[0:64, a:a + 512], start=True, stop=True)

            # copy conv bias (bf16, embedded in wbs) to fp32 on BOTH partition
            # halves (the ACT b-plane copy biases partitions 64-127)
            nc.vector.tensor_copy(out=convb128[0:64, :],
                                  in_=wb_sb[0:64, K * 384 + 40:K * 384 + 41])
            nc.vector.tensor_copy(out=convb128[64:128, :],
                                  in_=wb_sb[0:64, K * 384 + 40:K * 384 + 41])

            # fold partials: col sums and basis column 0
            nc.vector.tensor_reduce(out=M10[:, 3:4], in_=COLP[:, 0:7],
                                    axis=Ax.X, op=Alu.add)
            nc.vector.tensor_reduce(out=M10[:, 4:5], in_=COLP[:, 7:14],
                                    axis=Ax.X, op=Alu.add)
            nc.vector.tensor_reduce(out=M10[:, 0:1], in_=PARTS, axis=Ax.X, op=Alu.add)

            # per-channel coefficient contraction: G[c,k] = sum_b M10[c,b]*CW2[c,b,k]
            for k in range(K):
                nc.vector.scalar_tensor_tensor(
                    out=scr_g[:, 0:10], in0=M10[:, :], scalar=1.0,
                    in1=cw2v[:, :, k], op0=Alu.mult, op1=Alu.mult,
                    accum_out=G[:, k:k + 1])

            # logits broadcast (bf16 for a fast PE pass); exp + sum in one
            # ACT op; normalize on DVE
            nc.vector.tensor_copy(out=G_bf, in_=G)
            nc.tensor.matmul(psum_b, onesall, G_bf, start=True, stop=True)
            nc.scalar.activation(out=att_sb, in_=psum_b, func=Act.Exp,
                                 accum_out=rs_sum)
            # late upper-copy chunks issued from the scalar engine only after
            # its attention work (a dma_start blocks the engine if the hwdge
            # ring is full; by now the ring has drained)
            for a, ln in U_SCAL:
                nc.scalar.dma_start(out=XL[64:128, 2 + a:2 + a + ln],
                                    in_=xin[:, a:a + ln])
            nc.vector.reciprocal(out=rs_inv, in_=rs_sum)
            nc.vector.tensor_scalar_mul(out=attn, in0=att_sb, scalar1=rs_inv)

            # --- weight mixing: mw = sum_k attn_k * bank'_k, one group per
            # conv window so the first matmul unblocks early ---
            wv = wb_sb[:, 0:K * 384].rearrange("p (k c) -> p k c", k=K)
            mwf = mw.rearrange("p a b -> p (a b)")
            mbf = mwb.rearrange("p a b -> p (a b)")
            for g in range(3):
                sl = slice(g * 128, g * 128 + 128)
                nc.vector.tensor_scalar_mul(
                    out=mwf[:, sl], in0=wv[:, 0, sl], scalar1=attn[:, 0:1])
                for k in range(1, K):
                    tgt = mbf if k == K - 1 else mwf
                    nc.vector.scalar_tensor_tensor(
                        out=tgt[:, sl], in0=wv[:, k, sl],
                        scalar=attn[:, k:k + 1], in1=mwf[:, sl],
                        op0=Alu.mult, op1=Alu.add)

            # --- main conv: 33 dual-plane PSUM tiles (F=512, position-
            # indexed, not row-aligned) x 3 matmuls.  Output position
            # p = 131 + 130*row + (col+1); plane A of stream position p is
            # out(p), plane B is out(p+130).  ACT copies plane B into the
            # position-indexed bplane buffer; DVE does one STT per tile:
            # stg = (A + convb) + bplane[p]. ---
            P0 = 131
            PEND = P0 + H * WP
            NW = (PEND - P0 + 511) // 512        # 33 windows
            # window -> out-DMA group: trailing groups smaller so the final
            # DMA is short
            wgrp = []
            for g, size in enumerate((3, 3, 3, 3, 3, 3, 3, 3, 3, 2, 2, 2)):
                wgrp += [g] * size
            stg = None
            nc.vector.memset(bplane[:, 0:P0 + WP], 0.0)
            gbase = 0
            for t in range(NW):
                s = P0 + 512 * t
                F = min(512, PEND - s)
                pt = PS.tile([128, 512], f32, tag="cps", name=f"cps{t}")
                for w in range(3):
                    o = s + (w - 1)
                    nc.tensor.matmul(pt[:, :F], mwb[:, w, :], XL[:, o:o + F],
                                     start=(w == 0), stop=(w == 2))
                if t == 0 or wgrp[t] != wgrp[t - 1]:
                    stg = STG.tile([64, 3 * 512], bf16,
                                   tag="stg", name=f"stg{wgrp[t]}")
                    gbase = s
                # ACT: bplane[s+130 : s+130+F] = plane B
                nc.scalar.activation(out=bplane[:, s + WP:s + WP + F],
                                     in_=pt[64:128, :F], func=Act.Identity,
                                     bias=0.0, scale=1.0)
                # DVE: stg = (A + convb) + bplane[s:s+F]
                nc.vector.scalar_tensor_tensor(
                    out=stg[:, s - gbase:s - gbase + F], in0=pt[0:64, :F],
                    scalar=convb128[0:64, :], in1=bplane[:, s:s + F],
                    op0=Alu.add, op1=Alu.add)
                if t == NW - 1 or wgrp[t + 1] != wgrp[t]:
                    glen = s + F - gbase
                    eng = nc.sync if wgrp[t] % 2 == 1 else nc.scalar
                    eng.dma_start(out=outT[:, gbase:gbase + glen],
                                  in_=stg[:, :glen])

    nc.compile()
    return nc


def _get_nc():
    if "nc" not in _NC_CACHE:
        _NC_CACHE["nc"] = _build_nc()
    return _NC_CACHE["nc"]


def _prep_inputs(x, weight, conv_w, conv_b, net0_w, net0_b, net1_w, net1_b,
                 net2_w, net2_b):
    import ml_dtypes
    cw2 = _make_cw2(np.asarray(net0_w, np.float32), np.asarray(net0_b, np.float32),
                    np.asarray(net1_w, np.float32), np.asarray(net1_b, np.float32),
                    np.asarray(net2_w, np.float32), np.asarray(net2_b, np.float32))
    wsum = np.asarray(weight, np.float32) + np.asarray(conv_w, np.float32)[None]
    banks = np.stack([_make_bank3(wsum[k]) for k in range(K)])  # (K,128,3,128)
    bf = banks.reshape(K, 128, 384)
    wbs = np.zeros((128, K * 384 + 48), np.float32)
    for k in range(K):
        wbs[:, k * 384:(k + 1) * 384] = bf[k]
    wbs[0:64, K * 384:K * 384 + 40] = cw2.reshape(C, 40)
    wbs[0:64, K * 384 + 40] = np.asarray(conv_b, np.float32)
    wbs = np.ascontiguousarray(wbs).astype(ml_dtypes.bfloat16)
    x = np.asarray(x, np.float32)
    xp = np.zeros((N, C, H, WP), np.float32)
    xp[:, :, :, :W] = x
    xs = xp.astype(ml_dtypes.bfloat16)
    in_maps = []
    for n in range(N):
        in_maps.append({
            "xin": np.ascontiguousarray(xs[n].reshape(C, HWELEM)),
            "wbs": wbs,
        })
    return in_maps


def _run(inputs, trace=False, **kw):
    from concourse.bass_utils import run_bass_kernel_spmd
    nc = _get_nc()
    in_maps = _prep_inputs(**inputs)
    return run_bass_kernel_spmd(nc, in_maps, core_ids=list(range(N)), trace=trace, **kw)


def postprocess(raw):
    """raw: (N, C, 16772) flat position-indexed bf16/float -> (N,C,H,W)."""
    out = np.asarray(raw, np.float32)
    out = out[:, :, 131:131 + H * WP].reshape(out.shape[0], C, H, WP)
    return np.ascontiguousarray(out[:, :, :, 1:1 + W])


def kernel(**inputs):
    res = _run(inputs)
    raw = np.stack([np.asarray(res.results[n]["out"]) for n in range(N)])
    return postprocess(raw)
